# revision 15
# baseline (speedup 1.0000x reference)
"""CTC loss (Keras-style ctc_batch_cost) on Trainium2 via Bass/Tile.

Problem: B=256, T=512, C=128 (blank = C-1), U=128 labels per row, full
input/label lengths.  Output: loss [B, 1] f32.

Strategy
--------
Data-parallel over batch: 8 NeuronCores x 32 batch rows each.

Math: work in the *linear probability* domain with a constant per-step
rescale K (folded into the probabilities on the host), which turns the
log-domain alpha recursion into pure multiply-adds.  Split the extended
CTC state vector into label states a_t[u] (state 2u+1) and blank states
b_t[u] (state 2u):

    b_t[u] = pb_t * (b_{t-1}[u] + a_{t-1}[u-1])
    a_t[u] = pl_t[u] * (a_{t-1}[u] + b_{t-1}[u] + sk[u] * a_{t-1}[u-1])

For a fixed u, each of these is a first-order recurrence along t ->
one hardware `tensor_tensor_scan` instruction (state = (d0 + state) * d1)
over the whole time axis.  Per u-iteration: one scan for b, one fused
scalar_tensor_tensor to form c_t = b_{t-1}[u] + sk[u]*a_{t-1}[u-1], and
one scan for a.  129 iterations replace 511 sequential time steps.

The per-(b,t) label probabilities pl (a gather over classes) are computed
on the host (cheap numpy take_along_axis) and shipped pre-laid-out; this
keeps HBM traffic the same as shipping y_pred itself.

Layout: 128 SBUF partitions = 4 u-groups x 32 batch rows.  pl lives as
[128, U_local * T]; the recursion state arrays [128, T] hop partition
groups as u advances (3 one-off [32, T] copies).

loss = T*ln(K) - ln(a_T[U-1] + b_T[U]) computed on-device (ACT Ln).
"""

import numpy as np

B, T, C, U = 256, 512, 128, 128
BLANK = C - 1
NCORES = 8
BC = B // NCORES  # 32
K = 64.0
EPS = 1e-7

_cache = {}


def _build():
    import concourse.tile as tile
    from concourse import bacc, mybir

    F32 = mybir.dt.float32
    ADD = mybir.AluOpType.add
    MULT = mybir.AluOpType.mult
    LN = mybir.ActivationFunctionType.Ln
    COPY = mybir.ActivationFunctionType.Copy

    nc = bacc.Bacc("TRN2", target_bir_lowering=False)
    pl_d = nc.dram_tensor("pl", [128, 32 * T], F32, kind="ExternalInput")
    pb_d = nc.dram_tensor("pb", [128, T], F32, kind="ExternalInput")
    sk_d = nc.dram_tensor("sk", [128, 32], F32, kind="ExternalInput")
    out_d = nc.dram_tensor("loss", [BC, 1], F32, kind="ExternalOutput")

    with tile.TileContext(nc) as tc:
        with tc.tile_pool(name="pool", bufs=1) as pool:
            pl = pool.tile([128, 32 * T], F32, tag="pl")
            pb = pool.tile([128, T], F32, tag="pb")
            sk = pool.tile([128, 32], F32, tag="sk")
            Aa = pool.tile([128, T], F32, tag="Aa")
            Ab = pool.tile([128, T], F32, tag="Ab")
            Bb = pool.tile([128, T], F32, tag="Bb")
            Cc = pool.tile([128, T], F32, tag="Cc")
            fin = pool.tile([128, 4], F32, tag="fin")

            nc.sync.dma_start(pl[:], pl_d[:])
            nc.sync.dma_start(pb[:], pb_d[:])
            nc.sync.dma_start(sk[:], sk_d[:])

            # A_{-1} = 0 everywhere; init columns of the others
            nc.scalar.memzero(Ab[:])
            nc.scalar.memzero(Aa[:, 0:1])
            nc.scalar.memzero(Bb[:, 0:1])
            # t=0 initial values for u=0 (group 0)
            nc.scalar.copy(Bb[0:32, 0:1], pb[0:32, 0:1])

            abuf = [Aa, Ab]
            for u in range(U):
                g = u // 32
                j = u % 32
                p0, p1 = 32 * g, 32 * g + 32
                Aprev = abuf[(u + 1) % 2]
                Acur = abuf[u % 2]

                if u > 0 and j == 0:
                    # A_{u-1} hops from group g-1 partitions to group g
                    nc.scalar.copy(Aprev[p0:p1, :], Aprev[p0 - 32 : p0, :])

                if u == 1:
                    # B's t=0 column held b_0[0]=pb_0 for STT(0); from u=1 on
                    # it must be b_0[u]=0, and STT(1) reads it below
                    nc.scalar.memzero(Bb[0:32, 0:1])

                # scan1: B_u[t] = (A_{u-1}[t-1] + state) * pb[t]
                nc.vector.tensor_tensor_scan(
                    out=Bb[p0:p1, 1:T],
                    data0=Aprev[p0:p1, 0 : T - 1],
                    data1=pb[p0:p1, 1:T],
                    initial=pb[0:32, 0:1] if u == 0 else 0.0,
                    op0=ADD,
                    op1=MULT,
                )
                # c_t = sk[u]*A_{u-1}[t-1] + B_u[t-1]
                nc.vector.scalar_tensor_tensor(
                    out=Cc[p0:p1, 0 : T - 1],
                    in0=Aprev[p0:p1, 0 : T - 1],
                    scalar=sk[p0:p1, j : j + 1],
                    in1=Bb[p0:p1, 0 : T - 1],
                    op0=MULT,
                    op1=ADD,
                )
                # scan2: A_u[t] = (c_t + state) * pl[u][t]
                nc.vector.tensor_tensor_scan(
                    out=Acur[p0:p1, 1:T],
                    data0=Cc[p0:p1, 0 : T - 1],
                    data1=pl[p0:p1, j * T + 1 : j * T + T],
                    initial=pl[0:32, 0:1] if u == 0 else 0.0,
                    op0=ADD,
                    op1=MULT,
                )

                if u == 0:
                    # A_0[0] = pl[0,0]
                    nc.scalar.copy(Acur[0:32, 0:1], pl[0:32, 0:1])
                if u == 2:
                    # buffer that held A_0 gets reused; clear its t=0 column
                    nc.scalar.memzero(Acur[0:32, 0:1])

            # final blank row u = U: scan1 only (group 3 partitions)
            Aprev = abuf[(U + 1) % 2]  # A_{U-1}
            nc.vector.tensor_tensor_scan(
                out=Bb[96:128, 1:T],
                data0=Aprev[96:128, 0 : T - 1],
                data1=pb[96:128, 1:T],
                initial=0.0,
                op0=ADD,
                op1=MULT,
            )

            # loss = T*ln(K) - ln(a_fin + b_fin)
            nc.vector.tensor_add(
                fin[96:128, 0:1], Aprev[96:128, T - 1 : T], Bb[96:128, T - 1 : T]
            )
            nc.scalar.activation(fin[96:128, 1:2], fin[96:128, 0:1], func=LN)
            nc.scalar.activation(
                fin[96:128, 2:3],
                fin[96:128, 1:2],
                func=COPY,
                scale=-1.0,
                bias=float(T * np.log(K)),
            )
            nc.sync.dma_start(out_d[:], fin[96:128, 2:3])

    nc.compile()
    return nc


def _build_v2():
    """Wavefront-packed version: 128 partitions = 4 t-chunks x 32 batch.

    Chunk c covers recursion steps t in [128c+1, 128c+128] (t=512 is a
    padded fake step whose probabilities are 0; the answer is read at
    t=511 = col 127 of chunk 3).  Wave w computes cell (u = w - c,
    chunk c) for all four groups in one [128, 128] instruction; cells
    with u out of range compute exact zeros (pl padded to 0).

    Slabs A/B [128, 129]: col k of group c = value at t = 128c + k;
    col 0 is the chunk-boundary state, propagated between waves by
    3 x [32,1] partition-shifted copies on the Scalar engine (hidden
    under the DVE critical path).
    """
    import concourse.tile as tile
    from concourse import bacc, mybir

    F32 = mybir.dt.float32
    ADD = mybir.AluOpType.add
    MULT = mybir.AluOpType.mult
    LN = mybir.ActivationFunctionType.Ln
    COPY = mybir.ActivationFunctionType.Copy

    NW = 132  # waves: scan1 for w=0..131, STT/scan2 for w=0..130
    NWS = 131  # scan2/STT wave count (u <= 127 -> w <= 130)

    nc = bacc.Bacc("TRN2", target_bir_lowering=False)
    plw_d = nc.dram_tensor("plw", [128, NWS * 128], F32, kind="ExternalInput")
    pbc_d = nc.dram_tensor("pbc", [128, 128], F32, kind="ExternalInput")
    skw_d = nc.dram_tensor("skw", [128, NWS], F32, kind="ExternalInput")
    iw_d = nc.dram_tensor("iw", [128, 2], F32, kind="ExternalInput")
    out_d = nc.dram_tensor("loss", [BC, 2], F32, kind="ExternalOutput")

    PL_PIECES = [1, 3, 8, 16, 24, 32, 32, 15]  # waves per pl DMA piece (sums to 131)

    with tile.TileContext(nc) as tc:
        with tc.tile_pool(name="pool", bufs=1) as pool:
            plw = [
                pool.tile([128, n * 128], F32, tag=f"plw{i}", name=f"plw{i}")
                for i, n in enumerate(PL_PIECES)
            ]
            pbc = pool.tile([128, 128], F32, tag="pbc")
            skw = pool.tile([128, NWS], F32, tag="skw")
            iw = pool.tile([128, 2], F32, tag="iw")
            As = [pool.tile([128, 129], F32, tag=f"As{i}", name=f"As{i}") for i in range(2)]
            Bs = [pool.tile([128, 129], F32, tag=f"Bs{i}", name=f"Bs{i}") for i in range(2)]
            Cw = pool.tile([128, 128], F32, tag="Cw")
            fin = pool.tile([128, 4], F32, tag="fin")

            nc.sync.dma_start(pbc[:], pbc_d[:])
            nc.sync.dma_start(skw[:], skw_d[:])
            nc.sync.dma_start(iw[:], iw_d[:])
            # chain the pl pieces so piece 0 gets full DMA bandwidth and the
            # first waves can start within a few microseconds
            prev_dma = None
            from concourse.tile import add_dep_helper

            piece_start = [0]
            for n in PL_PIECES:
                piece_start.append(piece_start[-1] + n)
            for i, n in enumerate(PL_PIECES):
                lo = piece_start[i] * 128
                hi = piece_start[i + 1] * 128
                d = nc.sync.dma_start(plw[i][:, 0 : hi - lo], plw_d[:, lo:hi])
                if prev_dma is not None:
                    add_dep_helper(d.ins, prev_dma.ins, reason="serialize pl DMA")
                prev_dma = d

            # init: A_prev(-1) = 0 everywhere; boundary cols
            nc.scalar.memzero(As[1][:])
            nc.scalar.memzero(Bs[1][:, 0:1])
            nc.scalar.copy(As[0][:, 0:1], iw[:, 0:1])  # a0 at group 0
            nc.scalar.copy(Bs[0][:, 0:1], iw[:, 1:2])  # b0 at group 0

            def pl_ap(w):
                i = 0
                while w >= piece_start[i + 1]:
                    i += 1
                r = w - piece_start[i]
                return plw[i][:, r * 128 : (r + 1) * 128]

            for w in range(NW):
                Acur, Aprev = As[w % 2], As[(w + 1) % 2]
                Bcur = Bs[w % 2]
                Bnext = Bs[(w + 1) % 2]

                # scan1: B_u[t] = (A_{u-1}[t-1] + state) * pb[t]
                nc.vector.tensor_tensor_scan(
                    out=Bcur[:, 1:129],
                    data0=Aprev[:, 0:128],
                    data1=pbc[:],
                    initial=Bcur[:, 0:1],
                    op0=ADD,
                    op1=MULT,
                )
                # B boundary for wave w+1: can start right after scan1(w),
                # hides under STT+scan2 on DVE.  Split across GpSimd/ACT.
                if w < NW - 1:
                    nc.scalar.copy(Bnext[32:64, 0:1], Bcur[0:32, 128:129])
                    nc.gpsimd.tensor_copy(Bnext[64:96, 0:1], Bcur[32:64, 128:129])
                    nc.scalar.copy(Bnext[96:128, 0:1], Bcur[64:96, 128:129])
                if w < NWS:
                    # c_t = sk*A_{u-1}[t-1] + B_u[t-1]
                    nc.vector.scalar_tensor_tensor(
                        out=Cw[:],
                        in0=Aprev[:, 0:128],
                        scalar=skw[:, w : w + 1],
                        in1=Bcur[:, 0:128],
                        op0=MULT,
                        op1=ADD,
                    )
                    # scan2: A_u[t] = (c_t + state) * pl[u][t]
                    nc.vector.tensor_tensor_scan(
                        out=Acur[:, 1:129],
                        data0=Cw[:],
                        data1=pl_ap(w),
                        initial=Acur[:, 0:1],
                        op0=ADD,
                        op1=MULT,
                    )
                    # A boundary for wave w+1: after scan2(w), hides under
                    # scan1(w+1)+STT(w+1)
                    if w < NW - 1:
                        nc.gpsimd.tensor_copy(Aprev[32:64, 0:1], Acur[0:32, 128:129])
                        nc.scalar.copy(Aprev[64:96, 0:1], Acur[32:64, 128:129])
                        nc.gpsimd.tensor_copy(
                            Aprev[96:128, 0:1], Acur[64:96, 128:129]
                        )
                if w == 1:
                    # slabs As[0]/Bs[0] are reused at wave 2 with zero group-0
                    # boundary; wave 1's scan1/STT still read a_0[0] from
                    # As[0] col 0, so this must come after them (Tile orders
                    # the WAR hazard) and before wave 2
                    nc.gpsimd.memset(As[0][0:32, 0:1], 0.0)
                    nc.gpsimd.memset(Bs[0][0:32, 0:1], 0.0)

            # finals: A_127[511] (wave 130, As[0]); B_128[511] (wave 131, Bs[1])
            nc.vector.tensor_add(
                fin[96:128, 3:4],
                As[0][96:128, 127:128],
                Bs[1][96:128, 127:128],
            )
            nc.scalar.activation(fin[96:128, 1:2], fin[96:128, 3:4], func=LN)
            nc.scalar.activation(
                fin[96:128, 2:3],
                fin[96:128, 1:2],
                func=COPY,
                scale=-1.0,
                bias=float(T * np.log(K)),
            )
            nc.sync.dma_start(out_d[:], fin[96:128, 2:4])

    nc.compile()
    return nc


def _prep_core_v2(y_true_c, y_pred_c):
    """Host prep for the wavefront kernel."""
    NWS = 131
    yt = y_true_c.astype(np.int64)
    yp = y_pred_c.astype(np.float32)
    pl = np.take_along_axis(yp, yt[:, None, :], axis=2).astype(np.float32) + EPS
    plK = (pl * K).transpose(0, 2, 1)  # [32, U, T]
    plK_pad = np.zeros((BC, U, T + 1), np.float32)
    plK_pad[:, :, :T] = plK
    pbK = np.zeros((BC, T + 1), np.float32)
    pbK[:, :T] = (yp[:, :, BLANK].astype(np.float32) + EPS) * K

    sk = np.zeros((BC, U), np.float32)
    sk[:, 1:] = (yt[:, 1:] != yt[:, :-1]).astype(np.float32)

    plw = np.zeros((4, BC, NWS, 128), np.float32)
    skw = np.zeros((4, BC, NWS), np.float32)
    pbc = np.zeros((4, BC, 128), np.float32)
    for c in range(4):
        tsl = slice(128 * c + 1, 128 * c + 129)
        plw[c, :, c : c + U, :] = plK_pad[:, :, tsl]
        skw[c, :, c : c + U] = sk
        pbc[c] = pbK[:, tsl]

    iw = np.zeros((4, BC, 2), np.float32)
    iw[0, :, 0] = plK[:, 0, 0]
    iw[0, :, 1] = pbK[:, 0]

    return {
        "plw": np.ascontiguousarray(plw).reshape(128, NWS * 128),
        "pbc": pbc.reshape(128, 128),
        "skw": skw.reshape(128, NWS),
        "iw": iw.reshape(128, 2),
    }


def _prep_core(y_true_c, y_pred_c):
    """Host-side prep for one core's 32 batch rows -> input map dict."""
    yt = y_true_c.astype(np.int64)
    yp = y_pred_c.astype(np.float32)
    # label probs [32, T, U] -> [32, U, T]
    pl = np.take_along_axis(yp, yt[:, None, :], axis=2).astype(np.float32) + EPS
    pl = (pl * K).transpose(0, 2, 1)  # [32, U, T]
    # layout [128 = 4g x 32b, 32u, T]
    pl_sb = np.ascontiguousarray(
        pl.reshape(BC, 4, 32, T).transpose(1, 0, 2, 3)
    ).reshape(128, 32 * T)

    pb = (yp[:, :, BLANK].astype(np.float32) + EPS) * K  # [32, T]
    pb_sb = np.tile(pb, (4, 1))  # [128, T]

    sk = np.zeros((BC, U + 1), np.float32)
    sk[:, 1:U] = (yt[:, 1:] != yt[:, :-1]).astype(np.float32)
    sk_sb = np.zeros((128, 32), np.float32)
    for g in range(4):
        sk_sb[32 * g : 32 * g + 32, :] = sk[:, 32 * g : 32 * g + 32]

    return {"pl": pl_sb, "pb": pb_sb, "sk": sk_sb}


VERSION = 2


def _get_nc(version=None):
    v = VERSION if version is None else version
    key = f"nc{v}"
    if key not in _cache:
        _cache[key] = _build() if v == 1 else _build_v2()
    return _cache[key]


def _make_in_maps(y_true, y_pred, version=None):
    v = VERSION if version is None else version
    prep = _prep_core if v == 1 else _prep_core_v2
    return [
        prep(y_true[c * BC : (c + 1) * BC], y_pred[c * BC : (c + 1) * BC])
        for c in range(NCORES)
    ]


def kernel(y_true, y_pred):
    from concourse.bass_utils import run_bass_kernel_spmd

    y_true = np.asarray(y_true)
    y_pred = np.asarray(y_pred)
    nc = _get_nc()
    in_maps = _make_in_maps(y_true, y_pred)
    res = run_bass_kernel_spmd(nc, in_maps, core_ids=list(range(NCORES)))
    loss = np.concatenate(
        [res.results[c]["loss"][:, 0:1] for c in range(NCORES)], axis=0
    )
    return loss.astype(np.float32)


# revision 16
# speedup vs baseline: 1.0310x; 1.0310x over previous
"""CTC loss (Keras-style ctc_batch_cost) on Trainium2 via Bass/Tile.

Problem: B=256, T=512, C=128 (blank = C-1), U=128 labels per row, full
input/label lengths.  Output: loss [B, 1] f32.

Strategy
--------
Data-parallel over batch: 8 NeuronCores x 32 batch rows each.

Math: work in the *linear probability* domain with a constant per-step
rescale K (folded into the probabilities on the host), which turns the
log-domain alpha recursion into pure multiply-adds.  Split the extended
CTC state vector into label states a_t[u] (state 2u+1) and blank states
b_t[u] (state 2u):

    b_t[u] = pb_t * (b_{t-1}[u] + a_{t-1}[u-1])
    a_t[u] = pl_t[u] * (a_{t-1}[u] + b_{t-1}[u] + sk[u] * a_{t-1}[u-1])

For a fixed u, each of these is a first-order recurrence along t ->
one hardware `tensor_tensor_scan` instruction (state = (d0 + state) * d1)
over the whole time axis.  Per u-iteration: one scan for b, one fused
scalar_tensor_tensor to form c_t = b_{t-1}[u] + sk[u]*a_{t-1}[u-1], and
one scan for a.  129 iterations replace 511 sequential time steps.

The per-(b,t) label probabilities pl (a gather over classes) are computed
on the host (cheap numpy take_along_axis) and shipped pre-laid-out; this
keeps HBM traffic the same as shipping y_pred itself.

Layout: 128 SBUF partitions = 4 u-groups x 32 batch rows.  pl lives as
[128, U_local * T]; the recursion state arrays [128, T] hop partition
groups as u advances (3 one-off [32, T] copies).

loss = T*ln(K) - ln(a_T[U-1] + b_T[U]) computed on-device (ACT Ln).
"""

import numpy as np

B, T, C, U = 256, 512, 128, 128
BLANK = C - 1
NCORES = 8
BC = B // NCORES  # 32
K = 64.0
EPS = 1e-7

_cache = {}


def _build():
    import concourse.tile as tile
    from concourse import bacc, mybir

    F32 = mybir.dt.float32
    ADD = mybir.AluOpType.add
    MULT = mybir.AluOpType.mult
    LN = mybir.ActivationFunctionType.Ln
    COPY = mybir.ActivationFunctionType.Copy

    nc = bacc.Bacc("TRN2", target_bir_lowering=False)
    pl_d = nc.dram_tensor("pl", [128, 32 * T], F32, kind="ExternalInput")
    pb_d = nc.dram_tensor("pb", [128, T], F32, kind="ExternalInput")
    sk_d = nc.dram_tensor("sk", [128, 32], F32, kind="ExternalInput")
    out_d = nc.dram_tensor("loss", [BC, 1], F32, kind="ExternalOutput")

    with tile.TileContext(nc) as tc:
        with tc.tile_pool(name="pool", bufs=1) as pool:
            pl = pool.tile([128, 32 * T], F32, tag="pl")
            pb = pool.tile([128, T], F32, tag="pb")
            sk = pool.tile([128, 32], F32, tag="sk")
            Aa = pool.tile([128, T], F32, tag="Aa")
            Ab = pool.tile([128, T], F32, tag="Ab")
            Bb = pool.tile([128, T], F32, tag="Bb")
            Cc = pool.tile([128, T], F32, tag="Cc")
            fin = pool.tile([128, 4], F32, tag="fin")

            nc.sync.dma_start(pl[:], pl_d[:])
            nc.sync.dma_start(pb[:], pb_d[:])
            nc.sync.dma_start(sk[:], sk_d[:])

            # A_{-1} = 0 everywhere; init columns of the others
            nc.scalar.memzero(Ab[:])
            nc.scalar.memzero(Aa[:, 0:1])
            nc.scalar.memzero(Bb[:, 0:1])
            # t=0 initial values for u=0 (group 0)
            nc.scalar.copy(Bb[0:32, 0:1], pb[0:32, 0:1])

            abuf = [Aa, Ab]
            for u in range(U):
                g = u // 32
                j = u % 32
                p0, p1 = 32 * g, 32 * g + 32
                Aprev = abuf[(u + 1) % 2]
                Acur = abuf[u % 2]

                if u > 0 and j == 0:
                    # A_{u-1} hops from group g-1 partitions to group g
                    nc.scalar.copy(Aprev[p0:p1, :], Aprev[p0 - 32 : p0, :])

                if u == 1:
                    # B's t=0 column held b_0[0]=pb_0 for STT(0); from u=1 on
                    # it must be b_0[u]=0, and STT(1) reads it below
                    nc.scalar.memzero(Bb[0:32, 0:1])

                # scan1: B_u[t] = (A_{u-1}[t-1] + state) * pb[t]
                nc.vector.tensor_tensor_scan(
                    out=Bb[p0:p1, 1:T],
                    data0=Aprev[p0:p1, 0 : T - 1],
                    data1=pb[p0:p1, 1:T],
                    initial=pb[0:32, 0:1] if u == 0 else 0.0,
                    op0=ADD,
                    op1=MULT,
                )
                # c_t = sk[u]*A_{u-1}[t-1] + B_u[t-1]
                nc.vector.scalar_tensor_tensor(
                    out=Cc[p0:p1, 0 : T - 1],
                    in0=Aprev[p0:p1, 0 : T - 1],
                    scalar=sk[p0:p1, j : j + 1],
                    in1=Bb[p0:p1, 0 : T - 1],
                    op0=MULT,
                    op1=ADD,
                )
                # scan2: A_u[t] = (c_t + state) * pl[u][t]
                nc.vector.tensor_tensor_scan(
                    out=Acur[p0:p1, 1:T],
                    data0=Cc[p0:p1, 0 : T - 1],
                    data1=pl[p0:p1, j * T + 1 : j * T + T],
                    initial=pl[0:32, 0:1] if u == 0 else 0.0,
                    op0=ADD,
                    op1=MULT,
                )

                if u == 0:
                    # A_0[0] = pl[0,0]
                    nc.scalar.copy(Acur[0:32, 0:1], pl[0:32, 0:1])
                if u == 2:
                    # buffer that held A_0 gets reused; clear its t=0 column
                    nc.scalar.memzero(Acur[0:32, 0:1])

            # final blank row u = U: scan1 only (group 3 partitions)
            Aprev = abuf[(U + 1) % 2]  # A_{U-1}
            nc.vector.tensor_tensor_scan(
                out=Bb[96:128, 1:T],
                data0=Aprev[96:128, 0 : T - 1],
                data1=pb[96:128, 1:T],
                initial=0.0,
                op0=ADD,
                op1=MULT,
            )

            # loss = T*ln(K) - ln(a_fin + b_fin)
            nc.vector.tensor_add(
                fin[96:128, 0:1], Aprev[96:128, T - 1 : T], Bb[96:128, T - 1 : T]
            )
            nc.scalar.activation(fin[96:128, 1:2], fin[96:128, 0:1], func=LN)
            nc.scalar.activation(
                fin[96:128, 2:3],
                fin[96:128, 1:2],
                func=COPY,
                scale=-1.0,
                bias=float(T * np.log(K)),
            )
            nc.sync.dma_start(out_d[:], fin[96:128, 2:3])

    nc.compile()
    return nc


def _build_v2():
    """Wavefront-packed version: 128 partitions = 4 t-chunks x 32 batch.

    Chunk c covers recursion steps t in [128c+1, 128c+128] (t=512 is a
    padded fake step whose probabilities are 0; the answer is read at
    t=511 = col 127 of chunk 3).  Wave w computes cell (u = w - c,
    chunk c) for all four groups in one [128, 128] instruction; cells
    with u out of range compute exact zeros (pl padded to 0).

    Slabs A/B [128, 129]: col k of group c = value at t = 128c + k;
    col 0 is the chunk-boundary state, propagated between waves by
    3 x [32,1] partition-shifted copies on the Scalar engine (hidden
    under the DVE critical path).
    """
    import concourse.tile as tile
    from concourse import bacc, mybir

    F32 = mybir.dt.float32
    ADD = mybir.AluOpType.add
    MULT = mybir.AluOpType.mult
    LN = mybir.ActivationFunctionType.Ln
    COPY = mybir.ActivationFunctionType.Copy

    NW = 132  # waves: scan1 for w=0..131, STT/scan2 for w=0..130
    NWS = 131  # scan2/STT wave count (u <= 127 -> w <= 130)

    nc = bacc.Bacc("TRN2", target_bir_lowering=False)
    plw_d = nc.dram_tensor("plw", [128, NWS * 128], F32, kind="ExternalInput")
    pbc_d = nc.dram_tensor("pbc", [128, 128], F32, kind="ExternalInput")
    skw_d = nc.dram_tensor("skw", [128, NWS], F32, kind="ExternalInput")
    iw_d = nc.dram_tensor("iw", [128, 2], F32, kind="ExternalInput")
    out_d = nc.dram_tensor("loss", [BC, 2], F32, kind="ExternalOutput")

    PL_PIECES = [2, 6, 12, 20, 28, 32, 31]  # waves per pl DMA piece (sums to 131)

    with tile.TileContext(nc) as tc:
        with tc.tile_pool(name="pool", bufs=1) as pool:
            plw = [
                pool.tile([128, n * 128], F32, tag=f"plw{i}", name=f"plw{i}")
                for i, n in enumerate(PL_PIECES)
            ]
            pbc = pool.tile([128, 128], F32, tag="pbc")
            skw = pool.tile([128, NWS], F32, tag="skw")
            iw = pool.tile([128, 2], F32, tag="iw")
            As = [pool.tile([128, 129], F32, tag=f"As{i}", name=f"As{i}") for i in range(2)]
            Bs = [pool.tile([128, 129], F32, tag=f"Bs{i}", name=f"Bs{i}") for i in range(2)]
            Cw = pool.tile([128, 128], F32, tag="Cw")
            fin = pool.tile([128, 4], F32, tag="fin")

            nc.sync.dma_start(pbc[:], pbc_d[:])
            nc.sync.dma_start(skw[:], skw_d[:])
            nc.sync.dma_start(iw[:], iw_d[:])
            # chain the pl pieces so piece 0 gets full DMA bandwidth and the
            # first waves can start within a few microseconds
            prev_dma = None
            from concourse.tile import add_dep_helper

            piece_start = [0]
            for n in PL_PIECES:
                piece_start.append(piece_start[-1] + n)
            for i, n in enumerate(PL_PIECES):
                lo = piece_start[i] * 128
                hi = piece_start[i + 1] * 128
                d = nc.sync.dma_start(plw[i][:, 0 : hi - lo], plw_d[:, lo:hi])
                if prev_dma is not None:
                    add_dep_helper(d.ins, prev_dma.ins, reason="serialize pl DMA")
                prev_dma = d

            # init: A_prev(-1) = 0 everywhere; boundary cols
            nc.scalar.memzero(As[1][:])
            nc.scalar.memzero(Bs[1][:, 0:1])
            nc.scalar.copy(As[0][:, 0:1], iw[:, 0:1])  # a0 at group 0
            nc.scalar.copy(Bs[0][:, 0:1], iw[:, 1:2])  # b0 at group 0

            def pl_ap(w):
                i = 0
                while w >= piece_start[i + 1]:
                    i += 1
                r = w - piece_start[i]
                return plw[i][:, r * 128 : (r + 1) * 128]

            for w in range(NW):
                Acur, Aprev = As[w % 2], As[(w + 1) % 2]
                Bcur = Bs[w % 2]
                Bnext = Bs[(w + 1) % 2]

                # scan1: B_u[t] = (A_{u-1}[t-1] + state) * pb[t]
                nc.vector.tensor_tensor_scan(
                    out=Bcur[:, 1:129],
                    data0=Aprev[:, 0:128],
                    data1=pbc[:],
                    initial=Bcur[:, 0:1],
                    op0=ADD,
                    op1=MULT,
                )
                # B boundary for wave w+1: can start right after scan1(w),
                # hides under STT+scan2 on DVE.  Split across GpSimd/ACT.
                if w < NW - 1:
                    nc.scalar.copy(Bnext[32:64, 0:1], Bcur[0:32, 128:129])
                    nc.scalar.copy(Bnext[64:96, 0:1], Bcur[32:64, 128:129])
                    nc.scalar.copy(Bnext[96:128, 0:1], Bcur[64:96, 128:129])
                if w < NWS:
                    # c_t = sk*A_{u-1}[t-1] + B_u[t-1]
                    nc.vector.scalar_tensor_tensor(
                        out=Cw[:],
                        in0=Aprev[:, 0:128],
                        scalar=skw[:, w : w + 1],
                        in1=Bcur[:, 0:128],
                        op0=MULT,
                        op1=ADD,
                    )
                    # scan2: A_u[t] = (c_t + state) * pl[u][t]
                    nc.vector.tensor_tensor_scan(
                        out=Acur[:, 1:129],
                        data0=Cw[:],
                        data1=pl_ap(w),
                        initial=Acur[:, 0:1],
                        op0=ADD,
                        op1=MULT,
                    )
                    # A boundary for wave w+1: after scan2(w), hides under
                    # scan1(w+1)+STT(w+1)
                    if w < NW - 1:
                        nc.gpsimd.tensor_copy(Aprev[32:64, 0:1], Acur[0:32, 128:129])
                        nc.gpsimd.tensor_copy(Aprev[64:96, 0:1], Acur[32:64, 128:129])
                        nc.gpsimd.tensor_copy(
                            Aprev[96:128, 0:1], Acur[64:96, 128:129]
                        )
                if w == 1:
                    # slabs As[0]/Bs[0] are reused at wave 2 with zero group-0
                    # boundary; wave 1's scan1/STT still read a_0[0] from
                    # As[0] col 0, so this must come after them (Tile orders
                    # the WAR hazard) and before wave 2
                    nc.gpsimd.memset(As[0][0:32, 0:1], 0.0)
                    nc.gpsimd.memset(Bs[0][0:32, 0:1], 0.0)

            # finals: A_127[511] (wave 130, As[0]); B_128[511] (wave 131, Bs[1])
            nc.vector.tensor_add(
                fin[96:128, 3:4],
                As[0][96:128, 127:128],
                Bs[1][96:128, 127:128],
            )
            nc.scalar.activation(fin[96:128, 1:2], fin[96:128, 3:4], func=LN)
            nc.scalar.activation(
                fin[96:128, 2:3],
                fin[96:128, 1:2],
                func=COPY,
                scale=-1.0,
                bias=float(T * np.log(K)),
            )
            nc.sync.dma_start(out_d[:], fin[96:128, 2:4])

    nc.compile()
    return nc


def _prep_core_v2(y_true_c, y_pred_c):
    """Host prep for the wavefront kernel."""
    NWS = 131
    yt = y_true_c.astype(np.int64)
    yp = y_pred_c.astype(np.float32)
    pl = np.take_along_axis(yp, yt[:, None, :], axis=2).astype(np.float32) + EPS
    plK = (pl * K).transpose(0, 2, 1)  # [32, U, T]
    plK_pad = np.zeros((BC, U, T + 1), np.float32)
    plK_pad[:, :, :T] = plK
    pbK = np.zeros((BC, T + 1), np.float32)
    pbK[:, :T] = (yp[:, :, BLANK].astype(np.float32) + EPS) * K

    sk = np.zeros((BC, U), np.float32)
    sk[:, 1:] = (yt[:, 1:] != yt[:, :-1]).astype(np.float32)

    plw = np.zeros((4, BC, NWS, 128), np.float32)
    skw = np.zeros((4, BC, NWS), np.float32)
    pbc = np.zeros((4, BC, 128), np.float32)
    for c in range(4):
        tsl = slice(128 * c + 1, 128 * c + 129)
        plw[c, :, c : c + U, :] = plK_pad[:, :, tsl]
        skw[c, :, c : c + U] = sk
        pbc[c] = pbK[:, tsl]

    iw = np.zeros((4, BC, 2), np.float32)
    iw[0, :, 0] = plK[:, 0, 0]
    iw[0, :, 1] = pbK[:, 0]

    return {
        "plw": np.ascontiguousarray(plw).reshape(128, NWS * 128),
        "pbc": pbc.reshape(128, 128),
        "skw": skw.reshape(128, NWS),
        "iw": iw.reshape(128, 2),
    }


def _prep_core(y_true_c, y_pred_c):
    """Host-side prep for one core's 32 batch rows -> input map dict."""
    yt = y_true_c.astype(np.int64)
    yp = y_pred_c.astype(np.float32)
    # label probs [32, T, U] -> [32, U, T]
    pl = np.take_along_axis(yp, yt[:, None, :], axis=2).astype(np.float32) + EPS
    pl = (pl * K).transpose(0, 2, 1)  # [32, U, T]
    # layout [128 = 4g x 32b, 32u, T]
    pl_sb = np.ascontiguousarray(
        pl.reshape(BC, 4, 32, T).transpose(1, 0, 2, 3)
    ).reshape(128, 32 * T)

    pb = (yp[:, :, BLANK].astype(np.float32) + EPS) * K  # [32, T]
    pb_sb = np.tile(pb, (4, 1))  # [128, T]

    sk = np.zeros((BC, U + 1), np.float32)
    sk[:, 1:U] = (yt[:, 1:] != yt[:, :-1]).astype(np.float32)
    sk_sb = np.zeros((128, 32), np.float32)
    for g in range(4):
        sk_sb[32 * g : 32 * g + 32, :] = sk[:, 32 * g : 32 * g + 32]

    return {"pl": pl_sb, "pb": pb_sb, "sk": sk_sb}


VERSION = 2


def _get_nc(version=None):
    v = VERSION if version is None else version
    key = f"nc{v}"
    if key not in _cache:
        _cache[key] = _build() if v == 1 else _build_v2()
    return _cache[key]


def _make_in_maps(y_true, y_pred, version=None):
    v = VERSION if version is None else version
    prep = _prep_core if v == 1 else _prep_core_v2
    return [
        prep(y_true[c * BC : (c + 1) * BC], y_pred[c * BC : (c + 1) * BC])
        for c in range(NCORES)
    ]


def kernel(y_true, y_pred):
    from concourse.bass_utils import run_bass_kernel_spmd

    y_true = np.asarray(y_true)
    y_pred = np.asarray(y_pred)
    nc = _get_nc()
    in_maps = _make_in_maps(y_true, y_pred)
    res = run_bass_kernel_spmd(nc, in_maps, core_ids=list(range(NCORES)))
    loss = np.concatenate(
        [res.results[c]["loss"][:, 0:1] for c in range(NCORES)], axis=0
    )
    return loss.astype(np.float32)


# revision 17
# speedup vs baseline: 1.0418x; 1.0104x over previous
"""CTC loss (Keras-style ctc_batch_cost) on Trainium2 via Bass/Tile.

Problem: B=256, T=512, C=128 (blank = C-1), U=128 labels per row, full
input/label lengths.  Output: loss [B, 1] f32.

Strategy
--------
Data-parallel over batch: 8 NeuronCores x 32 batch rows each.

Math: work in the *linear probability* domain with a constant per-step
rescale K (folded into the probabilities on the host), which turns the
log-domain alpha recursion into pure multiply-adds.  Split the extended
CTC state vector into label states a_t[u] (state 2u+1) and blank states
b_t[u] (state 2u):

    b_t[u] = pb_t * (b_{t-1}[u] + a_{t-1}[u-1])
    a_t[u] = pl_t[u] * (a_{t-1}[u] + b_{t-1}[u] + sk[u] * a_{t-1}[u-1])

For a fixed u, each of these is a first-order recurrence along t ->
one hardware `tensor_tensor_scan` instruction (state = (d0 + state) * d1)
over the whole time axis.  Per u-iteration: one scan for b, one fused
scalar_tensor_tensor to form c_t = b_{t-1}[u] + sk[u]*a_{t-1}[u-1], and
one scan for a.  129 iterations replace 511 sequential time steps.

The per-(b,t) label probabilities pl (a gather over classes) are computed
on the host (cheap numpy take_along_axis) and shipped pre-laid-out; this
keeps HBM traffic the same as shipping y_pred itself.

Layout: 128 SBUF partitions = 4 u-groups x 32 batch rows.  pl lives as
[128, U_local * T]; the recursion state arrays [128, T] hop partition
groups as u advances (3 one-off [32, T] copies).

loss = T*ln(K) - ln(a_T[U-1] + b_T[U]) computed on-device (ACT Ln).
"""

import numpy as np

B, T, C, U = 256, 512, 128, 128
BLANK = C - 1
NCORES = 8
BC = B // NCORES  # 32
K = 64.0
EPS = 1e-7

_cache = {}


def _build():
    import concourse.tile as tile
    from concourse import bacc, mybir

    F32 = mybir.dt.float32
    ADD = mybir.AluOpType.add
    MULT = mybir.AluOpType.mult
    LN = mybir.ActivationFunctionType.Ln
    COPY = mybir.ActivationFunctionType.Copy

    nc = bacc.Bacc("TRN2", target_bir_lowering=False)
    pl_d = nc.dram_tensor("pl", [128, 32 * T], F32, kind="ExternalInput")
    pb_d = nc.dram_tensor("pb", [128, T], F32, kind="ExternalInput")
    sk_d = nc.dram_tensor("sk", [128, 32], F32, kind="ExternalInput")
    out_d = nc.dram_tensor("loss", [BC, 1], F32, kind="ExternalOutput")

    with tile.TileContext(nc) as tc:
        with tc.tile_pool(name="pool", bufs=1) as pool:
            pl = pool.tile([128, 32 * T], F32, tag="pl")
            pb = pool.tile([128, T], F32, tag="pb")
            sk = pool.tile([128, 32], F32, tag="sk")
            Aa = pool.tile([128, T], F32, tag="Aa")
            Ab = pool.tile([128, T], F32, tag="Ab")
            Bb = pool.tile([128, T], F32, tag="Bb")
            Cc = pool.tile([128, T], F32, tag="Cc")
            fin = pool.tile([128, 4], F32, tag="fin")

            nc.sync.dma_start(pl[:], pl_d[:])
            nc.sync.dma_start(pb[:], pb_d[:])
            nc.sync.dma_start(sk[:], sk_d[:])

            # A_{-1} = 0 everywhere; init columns of the others
            nc.scalar.memzero(Ab[:])
            nc.scalar.memzero(Aa[:, 0:1])
            nc.scalar.memzero(Bb[:, 0:1])
            # t=0 initial values for u=0 (group 0)
            nc.scalar.copy(Bb[0:32, 0:1], pb[0:32, 0:1])

            abuf = [Aa, Ab]
            for u in range(U):
                g = u // 32
                j = u % 32
                p0, p1 = 32 * g, 32 * g + 32
                Aprev = abuf[(u + 1) % 2]
                Acur = abuf[u % 2]

                if u > 0 and j == 0:
                    # A_{u-1} hops from group g-1 partitions to group g
                    nc.scalar.copy(Aprev[p0:p1, :], Aprev[p0 - 32 : p0, :])

                if u == 1:
                    # B's t=0 column held b_0[0]=pb_0 for STT(0); from u=1 on
                    # it must be b_0[u]=0, and STT(1) reads it below
                    nc.scalar.memzero(Bb[0:32, 0:1])

                # scan1: B_u[t] = (A_{u-1}[t-1] + state) * pb[t]
                nc.vector.tensor_tensor_scan(
                    out=Bb[p0:p1, 1:T],
                    data0=Aprev[p0:p1, 0 : T - 1],
                    data1=pb[p0:p1, 1:T],
                    initial=pb[0:32, 0:1] if u == 0 else 0.0,
                    op0=ADD,
                    op1=MULT,
                )
                # c_t = sk[u]*A_{u-1}[t-1] + B_u[t-1]
                nc.vector.scalar_tensor_tensor(
                    out=Cc[p0:p1, 0 : T - 1],
                    in0=Aprev[p0:p1, 0 : T - 1],
                    scalar=sk[p0:p1, j : j + 1],
                    in1=Bb[p0:p1, 0 : T - 1],
                    op0=MULT,
                    op1=ADD,
                )
                # scan2: A_u[t] = (c_t + state) * pl[u][t]
                nc.vector.tensor_tensor_scan(
                    out=Acur[p0:p1, 1:T],
                    data0=Cc[p0:p1, 0 : T - 1],
                    data1=pl[p0:p1, j * T + 1 : j * T + T],
                    initial=pl[0:32, 0:1] if u == 0 else 0.0,
                    op0=ADD,
                    op1=MULT,
                )

                if u == 0:
                    # A_0[0] = pl[0,0]
                    nc.scalar.copy(Acur[0:32, 0:1], pl[0:32, 0:1])
                if u == 2:
                    # buffer that held A_0 gets reused; clear its t=0 column
                    nc.scalar.memzero(Acur[0:32, 0:1])

            # final blank row u = U: scan1 only (group 3 partitions)
            Aprev = abuf[(U + 1) % 2]  # A_{U-1}
            nc.vector.tensor_tensor_scan(
                out=Bb[96:128, 1:T],
                data0=Aprev[96:128, 0 : T - 1],
                data1=pb[96:128, 1:T],
                initial=0.0,
                op0=ADD,
                op1=MULT,
            )

            # loss = T*ln(K) - ln(a_fin + b_fin)
            nc.vector.tensor_add(
                fin[96:128, 0:1], Aprev[96:128, T - 1 : T], Bb[96:128, T - 1 : T]
            )
            nc.scalar.activation(fin[96:128, 1:2], fin[96:128, 0:1], func=LN)
            nc.scalar.activation(
                fin[96:128, 2:3],
                fin[96:128, 1:2],
                func=COPY,
                scale=-1.0,
                bias=float(T * np.log(K)),
            )
            nc.sync.dma_start(out_d[:], fin[96:128, 2:3])

    nc.compile()
    return nc


def _build_v2():
    """Wavefront-packed version: 128 partitions = 4 t-chunks x 32 batch.

    Chunk c covers recursion steps t in [128c+1, 128c+128] (t=512 is a
    padded fake step whose probabilities are 0; the answer is read at
    t=511 = col 127 of chunk 3).  Wave w computes cell (u = w - c,
    chunk c) for all four groups in one [128, 128] instruction; cells
    with u out of range compute exact zeros (pl padded to 0).

    Slabs A/B [128, 129]: col k of group c = value at t = 128c + k;
    col 0 is the chunk-boundary state, propagated between waves by
    3 x [32,1] partition-shifted copies on the Scalar engine (hidden
    under the DVE critical path).
    """
    import concourse.tile as tile
    from concourse import bacc, mybir

    F32 = mybir.dt.float32
    ADD = mybir.AluOpType.add
    MULT = mybir.AluOpType.mult
    LN = mybir.ActivationFunctionType.Ln
    COPY = mybir.ActivationFunctionType.Copy

    NW = 132  # waves: scan1 for w=0..131, STT/scan2 for w=0..130
    NWS = 131  # scan2/STT wave count (u <= 127 -> w <= 130)

    nc = bacc.Bacc("TRN2", target_bir_lowering=False)
    plw_d = nc.dram_tensor("plw", [128, NWS * 128], F32, kind="ExternalInput")
    # aux = [pbc (128) | skw (NWS) | iw (2)] merged into one small DMA
    aux_d = nc.dram_tensor("aux", [128, 128 + NWS + 2], F32, kind="ExternalInput")
    out_d = nc.dram_tensor("loss", [BC, 2], F32, kind="ExternalOutput")

    PL_PIECES = [2, 6, 12, 20, 28, 32, 31]  # waves per pl DMA piece (sums to 131)

    with tile.TileContext(nc) as tc:
        with tc.tile_pool(name="pool", bufs=1) as pool:
            plw = [
                pool.tile([128, n * 128], F32, tag=f"plw{i}", name=f"plw{i}")
                for i, n in enumerate(PL_PIECES)
            ]
            aux = pool.tile([128, 128 + NWS + 2], F32, tag="aux")
            pbc = aux[:, 0:128]
            skw = aux[:, 128 : 128 + NWS]
            iw = aux[:, 128 + NWS : 128 + NWS + 2]
            As = [pool.tile([128, 129], F32, tag=f"As{i}", name=f"As{i}") for i in range(2)]
            Bs = [pool.tile([128, 129], F32, tag=f"Bs{i}", name=f"Bs{i}") for i in range(2)]
            Cw = pool.tile([128, 128], F32, tag="Cw")
            fin = pool.tile([128, 4], F32, tag="fin")

            nc.sync.dma_start(aux[:], aux_d[:])
            # chain the pl pieces so piece 0 gets full DMA bandwidth and the
            # first waves can start within a few microseconds
            prev_dma = None
            from concourse.tile import add_dep_helper

            piece_start = [0]
            for n in PL_PIECES:
                piece_start.append(piece_start[-1] + n)
            for i, n in enumerate(PL_PIECES):
                lo = piece_start[i] * 128
                hi = piece_start[i + 1] * 128
                d = nc.sync.dma_start(plw[i][:, 0 : hi - lo], plw_d[:, lo:hi])
                if prev_dma is not None:
                    add_dep_helper(d.ins, prev_dma.ins, reason="serialize pl DMA")
                prev_dma = d

            # init: A_prev(-1) = 0 everywhere; boundary cols
            nc.scalar.memzero(As[1][:])
            nc.scalar.memzero(Bs[1][:, 0:1])
            nc.scalar.copy(As[0][:, 0:1], iw[:, 0:1])  # a0 at group 0
            nc.scalar.copy(Bs[0][:, 0:1], iw[:, 1:2])  # b0 at group 0

            def pl_ap(w):
                i = 0
                while w >= piece_start[i + 1]:
                    i += 1
                r = w - piece_start[i]
                return plw[i][:, r * 128 : (r + 1) * 128]

            for w in range(NW):
                Acur, Aprev = As[w % 2], As[(w + 1) % 2]
                Bcur = Bs[w % 2]
                Bnext = Bs[(w + 1) % 2]

                # scan1: B_u[t] = (A_{u-1}[t-1] + state) * pb[t]
                nc.vector.tensor_tensor_scan(
                    out=Bcur[:, 1:129],
                    data0=Aprev[:, 0:128],
                    data1=pbc,
                    initial=Bcur[:, 0:1],
                    op0=ADD,
                    op1=MULT,
                )
                # B boundary for wave w+1: can start right after scan1(w),
                # hides under STT+scan2 on DVE.  Split across GpSimd/ACT.
                if w < NW - 1:
                    nc.scalar.copy(Bnext[32:64, 0:1], Bcur[0:32, 128:129])
                    nc.scalar.copy(Bnext[64:96, 0:1], Bcur[32:64, 128:129])
                    nc.scalar.copy(Bnext[96:128, 0:1], Bcur[64:96, 128:129])
                if w < NWS:
                    # c_t = sk*A_{u-1}[t-1] + B_u[t-1]
                    nc.vector.scalar_tensor_tensor(
                        out=Cw[:],
                        in0=Aprev[:, 0:128],
                        scalar=skw[:, w : w + 1],
                        in1=Bcur[:, 0:128],
                        op0=MULT,
                        op1=ADD,
                    )
                    # scan2: A_u[t] = (c_t + state) * pl[u][t]
                    nc.vector.tensor_tensor_scan(
                        out=Acur[:, 1:129],
                        data0=Cw[:],
                        data1=pl_ap(w),
                        initial=Acur[:, 0:1],
                        op0=ADD,
                        op1=MULT,
                    )
                    # A boundary for wave w+1: after scan2(w), hides under
                    # scan1(w+1)+STT(w+1)
                    if w < NW - 1:
                        nc.gpsimd.tensor_copy(Aprev[32:64, 0:1], Acur[0:32, 128:129])
                        nc.gpsimd.tensor_copy(Aprev[64:96, 0:1], Acur[32:64, 128:129])
                        nc.gpsimd.tensor_copy(
                            Aprev[96:128, 0:1], Acur[64:96, 128:129]
                        )
                if w == 1:
                    # slabs As[0]/Bs[0] are reused at wave 2 with zero group-0
                    # boundary; wave 1's scan1/STT still read a_0[0] from
                    # As[0] col 0, so this must come after them (Tile orders
                    # the WAR hazard) and before wave 2
                    nc.gpsimd.memset(As[0][0:32, 0:1], 0.0)
                    nc.gpsimd.memset(Bs[0][0:32, 0:1], 0.0)

            # finals: A_127[511] (wave 130, As[0]); B_128[511] (wave 131, Bs[1])
            nc.vector.tensor_add(
                fin[96:128, 3:4],
                As[0][96:128, 127:128],
                Bs[1][96:128, 127:128],
            )
            nc.scalar.activation(fin[96:128, 1:2], fin[96:128, 3:4], func=LN)
            nc.scalar.activation(
                fin[96:128, 2:3],
                fin[96:128, 1:2],
                func=COPY,
                scale=-1.0,
                bias=float(T * np.log(K)),
            )
            nc.sync.dma_start(out_d[:], fin[96:128, 2:4])

    nc.compile()
    return nc


def _prep_core_v2(y_true_c, y_pred_c):
    """Host prep for the wavefront kernel."""
    NWS = 131
    yt = y_true_c.astype(np.int64)
    yp = y_pred_c.astype(np.float32)
    pl = np.take_along_axis(yp, yt[:, None, :], axis=2).astype(np.float32) + EPS
    plK = (pl * K).transpose(0, 2, 1)  # [32, U, T]
    plK_pad = np.zeros((BC, U, T + 1), np.float32)
    plK_pad[:, :, :T] = plK
    pbK = np.zeros((BC, T + 1), np.float32)
    pbK[:, :T] = (yp[:, :, BLANK].astype(np.float32) + EPS) * K

    sk = np.zeros((BC, U), np.float32)
    sk[:, 1:] = (yt[:, 1:] != yt[:, :-1]).astype(np.float32)

    plw = np.zeros((4, BC, NWS, 128), np.float32)
    skw = np.zeros((4, BC, NWS), np.float32)
    pbc = np.zeros((4, BC, 128), np.float32)
    for c in range(4):
        tsl = slice(128 * c + 1, 128 * c + 129)
        plw[c, :, c : c + U, :] = plK_pad[:, :, tsl]
        skw[c, :, c : c + U] = sk
        pbc[c] = pbK[:, tsl]

    iw = np.zeros((4, BC, 2), np.float32)
    iw[0, :, 0] = plK[:, 0, 0]
    iw[0, :, 1] = pbK[:, 0]

    aux = np.concatenate(
        [pbc.reshape(128, 128), skw.reshape(128, NWS), iw.reshape(128, 2)], axis=1
    )
    return {
        "plw": np.ascontiguousarray(plw).reshape(128, NWS * 128),
        "aux": np.ascontiguousarray(aux),
    }


def _prep_core(y_true_c, y_pred_c):
    """Host-side prep for one core's 32 batch rows -> input map dict."""
    yt = y_true_c.astype(np.int64)
    yp = y_pred_c.astype(np.float32)
    # label probs [32, T, U] -> [32, U, T]
    pl = np.take_along_axis(yp, yt[:, None, :], axis=2).astype(np.float32) + EPS
    pl = (pl * K).transpose(0, 2, 1)  # [32, U, T]
    # layout [128 = 4g x 32b, 32u, T]
    pl_sb = np.ascontiguousarray(
        pl.reshape(BC, 4, 32, T).transpose(1, 0, 2, 3)
    ).reshape(128, 32 * T)

    pb = (yp[:, :, BLANK].astype(np.float32) + EPS) * K  # [32, T]
    pb_sb = np.tile(pb, (4, 1))  # [128, T]

    sk = np.zeros((BC, U + 1), np.float32)
    sk[:, 1:U] = (yt[:, 1:] != yt[:, :-1]).astype(np.float32)
    sk_sb = np.zeros((128, 32), np.float32)
    for g in range(4):
        sk_sb[32 * g : 32 * g + 32, :] = sk[:, 32 * g : 32 * g + 32]

    return {"pl": pl_sb, "pb": pb_sb, "sk": sk_sb}


VERSION = 2


def _get_nc(version=None):
    v = VERSION if version is None else version
    key = f"nc{v}"
    if key not in _cache:
        _cache[key] = _build() if v == 1 else _build_v2()
    return _cache[key]


def _make_in_maps(y_true, y_pred, version=None):
    v = VERSION if version is None else version
    prep = _prep_core if v == 1 else _prep_core_v2
    return [
        prep(y_true[c * BC : (c + 1) * BC], y_pred[c * BC : (c + 1) * BC])
        for c in range(NCORES)
    ]


def kernel(y_true, y_pred):
    from concourse.bass_utils import run_bass_kernel_spmd

    y_true = np.asarray(y_true)
    y_pred = np.asarray(y_pred)
    nc = _get_nc()
    in_maps = _make_in_maps(y_true, y_pred)
    res = run_bass_kernel_spmd(nc, in_maps, core_ids=list(range(NCORES)))
    loss = np.concatenate(
        [res.results[c]["loss"][:, 0:1] for c in range(NCORES)], axis=0
    )
    return loss.astype(np.float32)
